# revision 2
# baseline (speedup 1.0000x reference)
"""Trainium2 Bass kernel for nn_Lookback: causal running-mean over T.

out[b, t, c] = (1/(t+1)) * sum_{s<=t} x[b, s, c],  x: [8, 4096, 1024] fp32.

Sharding: data-parallel over batch B — core b handles x[b] ([4096, 1024]).

Per-core algorithm (T tiled into 32 blocks of P=128 rows):
  Phase A: tile column-sums  totals[j, c] = sum_p x_j[p, c]
           as one PSUM accumulation of 32 matmuls with indicator weights E_j
           (E_j[p, m] = [m == j], lhsT [128, 32]).
  Phase B: out_k = tril128 @ x_k + G_k @ totals
           where G_k[j, p] = [j < k] broadcasts the carry (sum of previous
           tile totals) to all 128 rows.  Both weights are 0/1 matrices.
  Scale by d[t] = 1/(t+1) during PSUM->SBUF eviction (per-partition scalar),
  then DMA to DRAM.

Matmuls use float32r (full fp32 data, 1 cycle/row at N>=256 vs 4 for fp32).
"""

import sys

import numpy as np

sys.path.insert(0, "/opt/trn_rl_repo")

import concourse.bass as bass
import concourse.mybir as mybir
import concourse.tile as tile
from concourse import bacc
from concourse.bass_utils import run_bass_kernel_spmd

B, T, C = 8, 4096, 1024
P = 128
NT = T // P          # 32 row tiles per core
CH = 512             # PSUM bank chunk (fp32)
NCH = C // CH
F32 = mybir.dt.float32
F32R = mybir.dt.float32r

_cache = {}


def _consts():
    """Host-precomputed weight matrices (shared by all cores)."""
    # trilT[q, p] = [q <= p]  (lhsT of the lower-triangular ones matrix)
    tril_t = np.tril(np.ones((P, P), np.float32)).T.copy()
    # E_all[:, k*NT:(k+1)*NT] = E_k with E_k[p, m] = [m == k]
    e_all = np.zeros((P, NT * NT), np.float32)
    for k in range(NT):
        e_all[:, k * NT + k] = 1.0
    # G_all[:, k*P:(k+1)*P] = G_k with G_k[j, p] = [j < k]
    g_all = np.zeros((NT, NT * P), np.float32)
    for k in range(NT):
        g_all[:k, k * P:(k + 1) * P] = 1.0
    # recip[p, k] = 1 / (128*k + p + 1)
    t_idx = np.arange(T, dtype=np.float64).reshape(NT, P).T  # [P, NT]
    recip = (1.0 / (t_idx + 1.0)).astype(np.float32)
    return tril_t, e_all, g_all, recip


def _build():
    nc = bacc.Bacc("TRN2", target_bir_lowering=False, debug=False, num_devices=B)
    x_d = nc.dram_tensor("x", [T, C], F32R, kind="ExternalInput").ap()
    tril_d = nc.dram_tensor("tril_t", [P, P], F32R, kind="ExternalInput").ap()
    e_d = nc.dram_tensor("e_all", [P, NT * NT], F32R, kind="ExternalInput").ap()
    g_d = nc.dram_tensor("g_all", [NT, NT * P], F32R, kind="ExternalInput").ap()
    r_d = nc.dram_tensor("recip", [P, NT], F32, kind="ExternalInput").ap()
    out_d = nc.dram_tensor("out", [T, C], F32, kind="ExternalOutput").ap()

    x_t = x_d.rearrange("(n p) c -> n p c", p=P)      # [NT, P, C]
    out_t = out_d.rearrange("(n p) c -> n p c", p=P)

    with tile.TileContext(nc) as tc:
        with (
            tc.tile_pool(name="const", bufs=1) as cp,
            tc.tile_pool(name="xres", bufs=1) as xp,
            tc.tile_pool(name="tot", bufs=1) as tp,
            tc.tile_pool(name="ev", bufs=4) as ep,
            tc.tile_pool(name="ps", bufs=3, space=bass.MemorySpace.PSUM) as psp,
            tc.tile_pool(name="pt", bufs=1, space=bass.MemorySpace.PSUM) as ptp,
        ):
            tril_s = cp.tile([P, P], F32R)
            e_s = cp.tile([P, NT * NT], F32R)
            g_s = cp.tile([NT, NT * P], F32R)
            r_s = cp.tile([P, NT], F32)
            nc.sync.dma_start(tril_s[:], tril_d)
            nc.sync.dma_start(e_s[:], e_d)
            nc.sync.dma_start(g_s[:], g_d)
            nc.sync.dma_start(r_s[:], r_d)

            xr = xp.tile([P, NT * C], F32R)            # resident input
            ptot = ptp.tile([NT, C], F32)             # totals accumulator
            tot_s = tp.tile([NT, C], F32R)

            # ---- load + phase A --------------------------------------
            for k in range(NT):
                xs = xr[:, k * C:(k + 1) * C]
                nc.sync.dma_start(xs, x_t[k])
                for h in range(NCH):
                    sl = slice(h * CH, (h + 1) * CH)
                    nc.tensor.matmul(
                        ptot[:, sl],
                        e_s[:, k * NT:(k + 1) * NT],
                        xs[:, sl],
                        start=(k == 0),
                        stop=(k == NT - 1),
                    )
            nc.vector.tensor_copy(tot_s[:], ptot[:])

            # ---- phase B + scaled eviction + store -------------------
            for k in range(NT):
                xs = xr[:, k * C:(k + 1) * C]
                ps = psp.tile([P, C], F32)
                for h in range(NCH):
                    sl = slice(h * CH, (h + 1) * CH)
                    nc.tensor.matmul(
                        ps[:, sl],
                        tril_s[:],
                        xs[:, sl],
                        start=True,
                        stop=(k == 0),
                    )
                    if k > 0:
                        nc.tensor.matmul(
                            ps[:, sl],
                            g_s[:, k * P:(k + 1) * P],
                            tot_s[:, sl],
                            start=False,
                            stop=True,
                        )
                o = ep.tile([P, C], F32)
                scale = r_s[:, k:k + 1]
                if k % 2 == 0:
                    nc.vector.tensor_scalar_mul(o[:], ps[:], scale)
                else:
                    nc.scalar.activation(
                        o[:], ps[:], mybir.ActivationFunctionType.Copy, scale=scale
                    )
                nc.sync.dma_start(out_t[k], o[:])

    nc.compile()
    return nc


def _run(x, trace=False):
    x = np.ascontiguousarray(x, dtype=np.float32)
    assert x.shape == (B, T, C)
    if "nc" not in _cache:
        _cache["nc"] = _build()
        _cache["consts"] = _consts()
    nc = _cache["nc"]
    tril_t, e_all, g_all, recip = _cache["consts"]
    in_maps = [
        {"x": x[b], "tril_t": tril_t, "e_all": e_all, "g_all": g_all, "recip": recip}
        for b in range(B)
    ]
    res = run_bass_kernel_spmd(nc, in_maps, core_ids=list(range(B)), trace=trace)
    out = np.stack([res.results[b]["out"] for b in range(B)])
    return out, res


def kernel(x):
    out, _ = _run(x, trace=False)
    return out


# revision 5
# speedup vs baseline: 1.1003x; 1.1003x over previous
"""Trainium2 Bass kernel for nn_Lookback: causal running-mean over T.

out[b, t, c] = (1/(t+1)) * sum_{s<=t} x[b, s, c],  x: [8, 4096, 1024] fp32.

Sharding: data-parallel over batch B — core b handles x[b] ([4096, 1024]).

Per-core algorithm (T tiled into 32 blocks of P=128 rows, pipelined as two
16-tile segments so segment 1's load/phase-A overlaps segment 0's phase B):
  Phase A: tile column-sums  totals[j, c] = sum_p x_j[p, c]
           as a PSUM accumulation of matmuls with indicator weights E_j.
  Phase B: out_k = tril128 @ x_k + G_k @ totals
           where G_k[j, p] = [j < k] broadcasts the carry (sum of previous
           tile totals) to all 128 rows.  Both weights are 0/1 matrices.
           totals rows of the not-yet-finished segment are zeros (memset),
           and G_k only weights rows j < k, so segment 0 outputs are exact.
  Scale by d[t] = 1/(t+1) during PSUM->SBUF eviction (per-partition scalar,
  alternating DVE / ACT), then DMA to DRAM.

Matmuls use float32r (fp32 bits, 1 cycle/row at N>=256 vs 4 for fp32).
"""

import sys

import numpy as np

sys.path.insert(0, "/opt/trn_rl_repo")

import concourse.bass as bass
import concourse.mybir as mybir
import concourse.tile as tile
from concourse import bacc
from concourse.bass_utils import run_bass_kernel_spmd

B, T, C = 8, 4096, 1024
P = 128
NT = T // P          # 32 row tiles per core
NSEG = 2
SEG = NT // NSEG     # 16 tiles per segment
CH = 512             # PSUM bank chunk (fp32)
NCH = C // CH
F32 = mybir.dt.float32
F32R = mybir.dt.float32r

_cache = {}


def _consts():
    """Host-precomputed weight matrices (shared by all cores)."""
    # trilT[q, p] = [q <= p]  (lhsT of the lower-triangular ones matrix)
    tril_t = np.tril(np.ones((P, P), np.float32)).T.copy()
    # E_all[:, k*NT:(k+1)*NT] = E_k with E_k[p, m] = [m == k] (global row)
    e_all = np.zeros((P, NT * NT), np.float32)
    for k in range(NT):
        e_all[:, k * NT + k] = 1.0
    # G_all[:, k*P:(k+1)*P] = G_k with G_k[j, p] = [j < k]
    g_all = np.zeros((NT, NT * P), np.float32)
    for k in range(NT):
        g_all[:k, k * P:(k + 1) * P] = 1.0
    # recip[p, k] = 1 / (128*k + p + 1)
    t_idx = np.arange(T, dtype=np.float64).reshape(NT, P).T  # [P, NT]
    recip = (1.0 / (t_idx + 1.0)).astype(np.float32)
    return tril_t, e_all, g_all, recip


def _build():
    nc = bacc.Bacc("TRN2", target_bir_lowering=False, debug=False, num_devices=B)
    x_d = nc.dram_tensor("x", [T, C], F32R, kind="ExternalInput").ap()
    tril_d = nc.dram_tensor("tril_t", [P, P], F32R, kind="ExternalInput").ap()
    e_d = nc.dram_tensor("e_all", [P, NT * NT], F32R, kind="ExternalInput").ap()
    g_d = nc.dram_tensor("g_all", [NT, NT * P], F32R, kind="ExternalInput").ap()
    r_d = nc.dram_tensor("recip", [P, NT], F32, kind="ExternalInput").ap()
    out_d = nc.dram_tensor("out", [T, C], F32, kind="ExternalOutput").ap()

    x_t = x_d.rearrange("(n p) c -> n p c", p=P)      # [NT, P, C]
    out_t = out_d.rearrange("(n p) c -> n p c", p=P)

    with tile.TileContext(nc) as tc:
        with (
            tc.tile_pool(name="const", bufs=1) as cp,
            tc.tile_pool(name="xres", bufs=1) as xp,
            tc.tile_pool(name="tot", bufs=1) as tp,
            tc.tile_pool(name="ev", bufs=4) as ep,
            tc.tile_pool(name="ps", bufs=2, space=bass.MemorySpace.PSUM) as psp,
            tc.tile_pool(name="pt", bufs=1, space=bass.MemorySpace.PSUM) as ptp,
        ):
            tril_s = cp.tile([P, P], F32R)
            e_s = cp.tile([P, NT * NT], F32R)
            g_s = cp.tile([NT, NT * P], F32R)
            r_s = cp.tile([P, NT], F32)
            nc.sync.dma_start(tril_s[:], tril_d)
            nc.sync.dma_start(e_s[:], e_d)
            nc.sync.dma_start(g_s[:], g_d)
            nc.sync.dma_start(r_s[:], r_d)

            xr = xp.tile([P, NT * C], F32R)           # resident input
            tot_s = tp.tile([NT, C], F32R)            # all tile totals
            pta = ptp.tile([NT, C], F32)
            ptb = ptp.tile([NT, C], F32)
            ptseg = [pta, ptb]

            for s in range(NSEG):
                k0, k1 = s * SEG, (s + 1) * SEG
                pt = ptseg[s]
                # ---- load + phase A for this segment -----------------
                for k in range(k0, k1):
                    xs = xr[:, k * C:(k + 1) * C]
                    nc.sync.dma_start(xs, x_t[k])
                    for h in range(NCH):
                        sl = slice(h * CH, (h + 1) * CH)
                        nc.tensor.matmul(
                            pt[:, sl],
                            e_s[:, k * NT:(k + 1) * NT],
                            xs[:, sl],
                            start=(k == k0),
                            stop=(k == k1 - 1),
                        )
                if s == 0:
                    nc.vector.tensor_copy(tot_s[:], pt[:])
                else:
                    nc.vector.tensor_add(tot_s[:], tot_s[:], pt[:])

                # ---- phase B + scaled eviction + store ---------------
                for k in range(k0, k1):
                    xs = xr[:, k * C:(k + 1) * C]
                    ps = psp.tile([P, C], F32)
                    # both chunks of the tril matmul first (same weights),
                    # then both chunks of the carry matmul
                    for h in range(NCH):
                        sl = slice(h * CH, (h + 1) * CH)
                        nc.tensor.matmul(
                            ps[:, sl], tril_s[:], xs[:, sl],
                            start=True, stop=(k == 0),
                        )
                    if k > 0:
                        for h in range(NCH):
                            sl = slice(h * CH, (h + 1) * CH)
                            nc.tensor.matmul(
                                ps[:, sl], g_s[:, k * P:(k + 1) * P], tot_s[:, sl],
                                start=False, stop=True,
                            )
                    o = ep.tile([P, C], F32)
                    scale = r_s[:, k:k + 1]
                    if k % 2 == 0:
                        nc.vector.tensor_scalar_mul(o[:], ps[:], scale)
                    else:
                        nc.scalar.activation(
                            o[:], ps[:], mybir.ActivationFunctionType.Copy,
                            scale=scale,
                        )
                    nc.sync.dma_start(out_t[k], o[:])

    nc.compile()
    return nc


def _run(x, trace=False):
    x = np.ascontiguousarray(x, dtype=np.float32)
    assert x.shape == (B, T, C)
    if "nc" not in _cache:
        _cache["nc"] = _build()
        _cache["consts"] = _consts()
    nc = _cache["nc"]
    tril_t, e_all, g_all, recip = _cache["consts"]
    in_maps = [
        {"x": x[b], "tril_t": tril_t, "e_all": e_all, "g_all": g_all, "recip": recip}
        for b in range(B)
    ]
    res = run_bass_kernel_spmd(nc, in_maps, core_ids=list(range(B)), trace=trace)
    out = np.stack([res.results[b]["out"] for b in range(B)])
    return out, res


def kernel(x):
    out, _ = _run(x, trace=False)
    return out


# revision 7
# speedup vs baseline: 1.1245x; 1.0220x over previous
"""Trainium2 Bass kernel for nn_Lookback: causal running-mean over T.

out[b, t, c] = (1/(t+1)) * sum_{s<=t} x[b, s, c],  x: [8, 4096, 1024] fp32.

Sharding: data-parallel over batch B — core b handles x[b] ([4096, 1024]).

Per-core algorithm (T tiled into 32 blocks of P=128 rows, pipelined as two
16-tile segments so segment 1's load/phase-A overlaps segment 0's phase B):
  Phase A: tile column-sums  totals[j, c] = sum_p x_j[p, c]
           as a PSUM accumulation of matmuls with indicator weights E_j.
  Phase B: out_k = tril128 @ x_k + G_k @ totals
           where G_k[j, p] = [j < k] broadcasts the carry (sum of previous
           tile totals) to all 128 rows.  Both weights are 0/1 matrices.
           totals rows of the not-yet-finished segment are zeros (memset),
           and G_k only weights rows j < k, so segment 0 outputs are exact.
  Scale by d[t] = 1/(t+1) during PSUM->SBUF eviction (per-partition scalar,
  alternating DVE / ACT), then DMA to DRAM.

Matmuls use float32r (fp32 bits, 1 cycle/row at N>=256 vs 4 for fp32).
"""

import sys

import numpy as np

sys.path.insert(0, "/opt/trn_rl_repo")

import concourse.bass as bass
import concourse.mybir as mybir
import concourse.tile as tile
from concourse import bacc
from concourse.bass_utils import run_bass_kernel_spmd

B, T, C = 8, 4096, 1024
P = 128
NT = T // P          # 32 row tiles per core
NSEG = 4
SEG = NT // NSEG     # 16 tiles per segment
CH = 512             # PSUM bank chunk (fp32)
NCH = C // CH
F32 = mybir.dt.float32
F32R = mybir.dt.float32r

_cache = {}


def _consts():
    """Host-precomputed weight matrices (shared by all cores)."""
    # trilT[q, p] = [q <= p]  (lhsT of the lower-triangular ones matrix)
    tril_t = np.tril(np.ones((P, P), np.float32)).T.copy()
    # E_all[:, k*NT:(k+1)*NT] = E_k with E_k[p, m] = [m == k] (global row)
    e_all = np.zeros((P, NT * NT), np.float32)
    for k in range(NT):
        e_all[:, k * NT + k] = 1.0
    # G_all[:, k*P:(k+1)*P] = G_k with G_k[j, p] = [j < k]
    g_all = np.zeros((NT, NT * P), np.float32)
    for k in range(NT):
        g_all[:k, k * P:(k + 1) * P] = 1.0
    # recip[p, k] = 1 / (128*k + p + 1)
    t_idx = np.arange(T, dtype=np.float64).reshape(NT, P).T  # [P, NT]
    recip = (1.0 / (t_idx + 1.0)).astype(np.float32)
    return tril_t, e_all, g_all, recip


def _build():
    nc = bacc.Bacc("TRN2", target_bir_lowering=False, debug=False, num_devices=B)
    x_d = nc.dram_tensor("x", [T, C], F32R, kind="ExternalInput").ap()
    tril_d = nc.dram_tensor("tril_t", [P, P], F32R, kind="ExternalInput").ap()
    e_d = nc.dram_tensor("e_all", [P, NT * NT], F32R, kind="ExternalInput").ap()
    g_d = nc.dram_tensor("g_all", [NT, NT * P], F32R, kind="ExternalInput").ap()
    r_d = nc.dram_tensor("recip", [P, NT], F32, kind="ExternalInput").ap()
    out_d = nc.dram_tensor("out", [T, C], F32, kind="ExternalOutput").ap()

    x_t = x_d.rearrange("(n p) c -> n p c", p=P)      # [NT, P, C]
    out_t = out_d.rearrange("(n p) c -> n p c", p=P)

    with tile.TileContext(nc) as tc:
        with (
            tc.tile_pool(name="const", bufs=1) as cp,
            tc.tile_pool(name="xres", bufs=1) as xp,
            tc.tile_pool(name="tot", bufs=1) as tp,
            tc.tile_pool(name="ev", bufs=4) as ep,
            tc.tile_pool(name="ps", bufs=2, space=bass.MemorySpace.PSUM) as psp,
            tc.tile_pool(name="pt", bufs=2, space=bass.MemorySpace.PSUM) as ptp,
        ):
            tril_s = cp.tile([P, P], F32R)
            e_s = cp.tile([P, NT * NT], F32R)
            g_s = cp.tile([NT, NT * P], F32R)
            r_s = cp.tile([P, NT], F32)
            nc.sync.dma_start(tril_s[:], tril_d)
            nc.sync.dma_start(e_s[:], e_d)
            nc.sync.dma_start(g_s[:], g_d)
            nc.sync.dma_start(r_s[:], r_d)

            xr = xp.tile([P, NT * C], F32R)           # resident input
            tot_list = []

            for s in range(NSEG):
                k0, k1 = s * SEG, (s + 1) * SEG
                pt = ptp.tile([NT, C], F32)
                # ---- load + phase A for this segment -----------------
                for k in range(k0, k1):
                    xs = xr[:, k * C:(k + 1) * C]
                    nc.sync.dma_start(xs, x_t[k])
                    for h in range(NCH):
                        sl = slice(h * CH, (h + 1) * CH)
                        nc.tensor.matmul(
                            pt[:, sl],
                            e_s[:, k * NT:(k + 1) * NT],
                            xs[:, sl],
                            start=(k == k0),
                            stop=(k == k1 - 1),
                        )
                # per-segment running totals tile: no WAR against the G
                # matmuls of earlier segments (they read their own tile)
                tot_s = tp.tile([NT, C], F32R, tag=f"tot{s}")
                if s == 0:
                    nc.vector.tensor_copy(tot_s[:], pt[:])
                else:
                    nc.vector.tensor_add(tot_s[:], tot_list[s - 1][:], pt[:])
                tot_list.append(tot_s)

                # ---- phase B + scaled eviction + store ---------------
                for k in range(k0, k1):
                    xs = xr[:, k * C:(k + 1) * C]
                    ps = psp.tile([P, C], F32)
                    # both chunks of the tril matmul first (same weights),
                    # then both chunks of the carry matmul
                    for h in range(NCH):
                        sl = slice(h * CH, (h + 1) * CH)
                        nc.tensor.matmul(
                            ps[:, sl], tril_s[:], xs[:, sl],
                            start=True, stop=(k == 0),
                        )
                    if k > 0:
                        for h in range(NCH):
                            sl = slice(h * CH, (h + 1) * CH)
                            nc.tensor.matmul(
                                ps[:, sl], g_s[:, k * P:(k + 1) * P], tot_s[:, sl],
                                start=False, stop=True,
                            )
                    o = ep.tile([P, C], F32)
                    scale = r_s[:, k:k + 1]
                    if k % 2 == 0:
                        nc.vector.tensor_scalar_mul(o[:], ps[:], scale)
                    else:
                        nc.scalar.activation(
                            o[:], ps[:], mybir.ActivationFunctionType.Copy,
                            scale=scale,
                        )
                    nc.sync.dma_start(out_t[k], o[:])

    nc.compile()
    return nc


def _run(x, trace=False):
    x = np.ascontiguousarray(x, dtype=np.float32)
    assert x.shape == (B, T, C)
    if "nc" not in _cache:
        _cache["nc"] = _build()
        _cache["consts"] = _consts()
    nc = _cache["nc"]
    tril_t, e_all, g_all, recip = _cache["consts"]
    in_maps = [
        {"x": x[b], "tril_t": tril_t, "e_all": e_all, "g_all": g_all, "recip": recip}
        for b in range(B)
    ]
    res = run_bass_kernel_spmd(nc, in_maps, core_ids=list(range(B)), trace=trace)
    out = np.stack([res.results[b]["out"] for b in range(B)])
    return out, res


def kernel(x):
    out, _ = _run(x, trace=False)
    return out


# revision 9
# speedup vs baseline: 1.1724x; 1.0426x over previous
"""Trainium2 Bass kernel for nn_Lookback: causal running-mean over T.

out[b, t, c] = (1/(t+1)) * sum_{s<=t} x[b, s, c],  x: [8, 4096, 1024] fp32.

Sharding: data-parallel over batch B — core b handles x[b] ([4096, 1024]).

Per-core algorithm (T tiled into 32 blocks of P=128 rows, pipelined as two
16-tile segments so segment 1's load/phase-A overlaps segment 0's phase B):
  Phase A: tile column-sums  totals[j, c] = sum_p x_j[p, c]
           as a PSUM accumulation of matmuls with indicator weights E_j.
  Phase B: out_k = tril128 @ x_k + G_k @ totals
           where G_k[j, p] = [j < k] broadcasts the carry (sum of previous
           tile totals) to all 128 rows.  Both weights are 0/1 matrices.
           totals rows of the not-yet-finished segment are zeros (memset),
           and G_k only weights rows j < k, so segment 0 outputs are exact.
  Scale by d[t] = 1/(t+1) during PSUM->SBUF eviction (per-partition scalar,
  alternating DVE / ACT), then DMA to DRAM.

Matmuls use float32r (fp32 bits, 1 cycle/row at N>=256 vs 4 for fp32).
"""

import sys

import numpy as np

sys.path.insert(0, "/opt/trn_rl_repo")

import concourse.bass as bass
import concourse.mybir as mybir
import concourse.tile as tile
from concourse import bacc
from concourse.bass_utils import run_bass_kernel_spmd

B, T, C = 8, 4096, 1024
P = 128
NT = T // P          # 32 row tiles per core
NSEG = 4
SEG = NT // NSEG     # 16 tiles per segment
CH = 512             # PSUM bank chunk (fp32)
NCH = C // CH
F32 = mybir.dt.float32
F32R = mybir.dt.float32r

_cache = {}


def _consts():
    """Host-precomputed weight matrices (shared by all cores)."""
    # trilT[q, p] = [q <= p]  (lhsT of the lower-triangular ones matrix)
    tril_t = np.tril(np.ones((P, P), np.float32)).T.copy()
    # E_all[:, k*NT:(k+1)*NT] = E_k with E_k[p, m] = [m == k] (global row)
    e_all = np.zeros((P, NT * NT), np.float32)
    for k in range(NT):
        e_all[:, k * NT + k] = 1.0
    # G_all[:, k*P:(k+1)*P] = G_k with G_k[j, p] = [j < k]
    g_all = np.zeros((NT, NT * P), np.float32)
    for k in range(NT):
        g_all[:k, k * P:(k + 1) * P] = 1.0
    # recip[p, k] = 1 / (128*k + p + 1)
    t_idx = np.arange(T, dtype=np.float64).reshape(NT, P).T  # [P, NT]
    recip = (1.0 / (t_idx + 1.0)).astype(np.float32)
    return tril_t, e_all, g_all, recip


def _build():
    nc = bacc.Bacc("TRN2", target_bir_lowering=False, debug=False, num_devices=B)
    x_d = nc.dram_tensor("x", [T, C], F32R, kind="ExternalInput").ap()
    tril_d = nc.dram_tensor("tril_t", [P, P], F32R, kind="ExternalInput").ap()
    e_d = nc.dram_tensor("e_all", [P, NT * NT], F32R, kind="ExternalInput").ap()
    g_d = nc.dram_tensor("g_all", [NT, NT * P], F32R, kind="ExternalInput").ap()
    r_d = nc.dram_tensor("recip", [P, NT], F32, kind="ExternalInput").ap()
    out_d = nc.dram_tensor("out", [T, C], F32, kind="ExternalOutput").ap()

    x_t = x_d.rearrange("(n p) c -> n p c", p=P)      # [NT, P, C]
    out_t = out_d.rearrange("(n p) c -> n p c", p=P)

    with tile.TileContext(nc) as tc:
        with (
            tc.tile_pool(name="const", bufs=1) as cp,
            tc.tile_pool(name="xres", bufs=1) as xp,
            tc.tile_pool(name="tot", bufs=1) as tp,
            tc.tile_pool(name="ev", bufs=4) as ep,
            tc.tile_pool(name="ps", bufs=3, space=bass.MemorySpace.PSUM) as psp,
            tc.tile_pool(name="pt", bufs=1, space=bass.MemorySpace.PSUM) as ptp,
        ):
            tril_s = cp.tile([P, P], F32R)
            e_s = cp.tile([P, NT * NT], F32R)
            g_s = cp.tile([NT, NT * P], F32R)
            r_s = cp.tile([P, NT], F32)
            nc.sync.dma_start(tril_s[:], tril_d)
            nc.sync.dma_start(e_s[:], e_d)
            nc.sync.dma_start(g_s[:], g_d)
            nc.sync.dma_start(r_s[:], r_d)

            xr = xp.tile([P, NT * C], F32R)           # resident input
            tot_list = []

            # PE warm-up burst: ~10us of back-to-back dummy matmuls while
            # the first segment streams in, so the HAM clock gate reaches
            # 8/8 (2.4 GHz) before the real matmul streams start.
            dmy = psp.tile([P, CH], F32, tag="ps")
            for _ in range(40):
                nc.tensor.matmul(dmy[:], tril_s[:], e_s[:, 0:CH],
                                 start=True, stop=True)

            for s in range(NSEG):
                k0, k1 = s * SEG, (s + 1) * SEG
                pt = ptp.tile([NT, C], F32)
                # ---- load + phase A for this segment -----------------
                for k in range(k0, k1):
                    xs = xr[:, k * C:(k + 1) * C]
                    nc.sync.dma_start(xs, x_t[k])
                    for h in range(NCH):
                        sl = slice(h * CH, (h + 1) * CH)
                        nc.tensor.matmul(
                            pt[:, sl],
                            e_s[:, k * NT:(k + 1) * NT],
                            xs[:, sl],
                            start=(k == k0),
                            stop=(k == k1 - 1),
                        )
                # per-segment running totals tile: no WAR against the G
                # matmuls of earlier segments (they read their own tile)
                tot_s = tp.tile([NT, C], F32R, tag=f"tot{s}")
                if s == 0:
                    nc.vector.tensor_copy(tot_s[:], pt[:])
                else:
                    nc.vector.tensor_add(tot_s[:], tot_list[s - 1][:], pt[:])
                tot_list.append(tot_s)

                # ---- phase B + scaled eviction + store ---------------
                for k in range(k0, k1):
                    xs = xr[:, k * C:(k + 1) * C]
                    ps = psp.tile([P, C], F32)
                    # both chunks of the tril matmul first (same weights),
                    # then both chunks of the carry matmul
                    for h in range(NCH):
                        sl = slice(h * CH, (h + 1) * CH)
                        nc.tensor.matmul(
                            ps[:, sl], tril_s[:], xs[:, sl],
                            start=True, stop=(k == 0),
                        )
                    if k > 0:
                        for h in range(NCH):
                            sl = slice(h * CH, (h + 1) * CH)
                            nc.tensor.matmul(
                                ps[:, sl], g_s[:, k * P:(k + 1) * P], tot_s[:, sl],
                                start=False, stop=True,
                            )
                    o = ep.tile([P, C], F32)
                    scale = r_s[:, k:k + 1]
                    if k % 2 == 0:
                        nc.vector.tensor_scalar_mul(o[:], ps[:], scale)
                    else:
                        nc.scalar.activation(
                            o[:], ps[:], mybir.ActivationFunctionType.Copy,
                            scale=scale,
                        )
                    nc.sync.dma_start(out_t[k], o[:])

    nc.compile()
    return nc


def _run(x, trace=False):
    x = np.ascontiguousarray(x, dtype=np.float32)
    assert x.shape == (B, T, C)
    if "nc" not in _cache:
        _cache["nc"] = _build()
        _cache["consts"] = _consts()
    nc = _cache["nc"]
    tril_t, e_all, g_all, recip = _cache["consts"]
    in_maps = [
        {"x": x[b], "tril_t": tril_t, "e_all": e_all, "g_all": g_all, "recip": recip}
        for b in range(B)
    ]
    res = run_bass_kernel_spmd(nc, in_maps, core_ids=list(range(B)), trace=trace)
    out = np.stack([res.results[b]["out"] for b in range(B)])
    return out, res


def kernel(x):
    out, _ = _run(x, trace=False)
    return out
